# revision 16
# baseline (speedup 1.0000x reference)
"""Trainium2 Bass kernel for nn_Attention_Module (submitter/reviewer attention pooling).

Reference math:
    Q  = submitter_emb @ Wq.T + bq                      [B, A]
    K  = einsum('blt,at->bla', reviewer_emb, Wk) + bk   [B, L, A]
    e  = einsum('ba,bla->bl', Q, K) / sqrt(A)           [B, L]
    ww = einsum('bl,blt->bt', e, reviewer_emb)          [B, T]

Algebraic rewrite used here (exact, just reassociation):
    qt[b, t] = sum_a Q[b, a] * Wk[a, t]        (= Q @ Wk,   [B, T], tiny)
    cb[b]    = sum_a Q[b, a] * bk[a]           (scalar per batch)
    e[b, l]  = (reviewer[b, l, :] . qt[b, :] + cb[b]) / sqrt(A)
    ww[b, t] = sum_l e[b, l] * reviewer[b, l, t]

This collapses the 137-GFLOP K matmul into a single streaming pass over
reviewer_emb: one fused DVE multiply+reduce (scalar_tensor_tensor with
accum_out) per tile for e, and PE matmuls with e as the stationary
operand (lhsT [128,1]) streaming the reviewer tile as the moving
operand, accumulating ww as a [1, 1024] PSUM row.

Sharding: data-parallel over batch B=32 -> 4 batches per core x 8 cores.
Weights replicated.  No cross-core communication; host concatenates.

Host-side prep is layout-only (transpose/reshape of inputs for DMA
efficiency); all input-dependent arithmetic runs on device in fp32.
"""

import numpy as np

import concourse.bass as bass
import concourse.bacc as bacc
import concourse.tile as tile
from concourse import mybir
from concourse.bass_utils import run_bass_kernel_spmd

# Problem shapes (hardcoded per contract)
B, L, T, A = 32, 2048, 1024, 1024
NCORES = 8
BLOC = B // NCORES          # 4 batches per core
P = 128                     # partitions
NCH = T // P                # 8 chunks of 128 along T/A
LTILE = 4                   # reviewer rows per partition per DMA tile
ROWS_PER_TILE = P * LTILE   # 512 rows -> 2 MiB per DMA
NTI = L // ROWS_PER_TILE    # 4 DMA tiles per batch
SCALE = 1.0 / float(np.sqrt(A))

F32 = mybir.dt.float32
FT = mybir.ActivationFunctionType
OP = mybir.AluOpType


def _build():
    nc = bacc.Bacc("TRN2", target_bir_lowering=False, debug=False, num_devices=NCORES)

    # subt: submitter slice pre-tiled to [128, NCH, BLOC] (t-major on partitions)
    subt = nc.dram_tensor("subt", [P, NCH, BLOC], F32, kind="ExternalInput").ap()
    rev = nc.dram_tensor("rev", [BLOC, L, T], F32, kind="ExternalInput").ap()
    # wqt: Wq.T, [T, A] row-major
    wqt = nc.dram_tensor("wqt", [T, A], F32, kind="ExternalInput").ap()
    # bqc/bkc: biases chunked [128, NCH] with element (p, c) = bias[128c + p]
    bqc = nc.dram_tensor("bqc", [P, NCH], F32, kind="ExternalInput").ap()
    wk = nc.dram_tensor("wk", [A, T], F32, kind="ExternalInput").ap()
    bkc = nc.dram_tensor("bkc", [P, NCH], F32, kind="ExternalInput").ap()
    ident = nc.dram_tensor("ident", [BLOC, BLOC], F32, kind="ExternalInput").ap()
    out = nc.dram_tensor("out", [BLOC, T], F32, kind="ExternalOutput").ap()

    with tile.TileContext(nc) as tc:
        with (
            tc.tile_pool(name="small", bufs=1) as small,
            tc.tile_pool(name="wqtp", bufs=1) as wqtp,
            tc.tile_pool(name="wkp", bufs=1) as wkp,
            tc.tile_pool(name="qb", bufs=1) as qbp,
            tc.tile_pool(name="rp", bufs=6) as rp,
            tc.tile_pool(name="scr", bufs=3) as scrp,
            tc.tile_pool(name="ep", bufs=4) as ep,
            tc.tile_pool(name="wwp", bufs=2) as wwp,
            tc.tile_pool(name="dram", bufs=1, space="DRAM") as dram,
            tc.tile_pool(name="psQ", bufs=1, space="PSUM") as psQp,
            tc.tile_pool(name="pstr", bufs=2, space="PSUM") as pstrp,
            tc.tile_pool(name="psq", bufs=1, space="PSUM") as psqp,
            tc.tile_pool(name="psww", bufs=1, space="PSUM") as pswwp,
        ):
            # ---- small loads (already laid out by host) ----
            subt_sb = small.tile([P, NCH, BLOC], F32, name="subt_sb", tag="subt_sb")
            nc.sync.dma_start(out=subt_sb, in_=subt)
            bq_sb = small.tile([P, NCH], F32, name="bq_sb", tag="bq_sb")
            nc.sync.dma_start(out=bq_sb, in_=bqc)
            bk_sb = small.tile([P, NCH], F32, name="bk_sb", tag="bk_sb")
            nc.sync.dma_start(out=bk_sb, in_=bkc)
            id_sb = small.tile([BLOC, BLOC], F32, name="id_sb", tag="id_sb")
            nc.sync.dma_start(out=id_sb, in_=ident)

            # ---- prefetch the first reviewer tile ahead of the weight wave
            #      so the main loop isn't DMA-gated at its start
            rt_pre = []
            for k in range(2):
                rt_k = rp.tile([P, LTILE, T], F32, name="rt", tag="rt")
                nc.sync.dma_start(
                    out=rt_k,
                    in_=rev[
                        0, k * ROWS_PER_TILE : (k + 1) * ROWS_PER_TILE, :
                    ].rearrange("(p f) t -> p f t", f=LTILE),
                )
                rt_pre.append(rt_k)

            # ---- weight loads: wqT[j] = [128 t, 1024 a], wk[i] = [128 a, 1024 t] ----
            wqT = [
                wqtp.tile([P, A], F32, name=f"wqT{j}", tag=f"wqT{j}")
                for j in range(NCH)
            ]
            for j in range(NCH):
                nc.sync.dma_start(out=wqT[j], in_=wqt[j * P : (j + 1) * P, :])
            wk_sb = [
                wkp.tile([P, T], F32, name=f"wk{i}", tag=f"wk{i}") for i in range(NCH)
            ]
            for i in range(NCH):
                nc.sync.dma_start(out=wk_sb[i], in_=wk[i * P : (i + 1) * P, :])

            # ---- Q[b, a] = s @ Wq.T : Wq.T streams as the moving operand,
            #      so these matmuls overlap the weight-DMA wave chunk by chunk.
            psQ = psQp.tile([BLOC, A], F32, name="psQ", tag="psQ")
            for j in range(NCH):
                for h in range(2):
                    nc.tensor.matmul(
                        psQ[:, h * 512 : (h + 1) * 512],
                        subt_sb[:, j, :],
                        wqT[j][:, h * 512 : (h + 1) * 512],
                        start=(j == 0),
                        stop=(j == NCH - 1),
                    )
            Q_sb = small.tile([BLOC, A], F32, name="Q_sb", tag="Q_sb")
            nc.scalar.copy(Q_sb, psQ)

            # ---- QT chunks [128 a, BLOC] via tiny PE transposes; add bq here ----
            QT = small.tile([P, NCH, BLOC], F32, name="QT", tag="QT")
            for i in range(NCH):
                pstr = pstrp.tile([P, BLOC], F32, name="pstr", tag="pstr")
                nc.tensor.transpose(pstr, Q_sb[:, i * P : (i + 1) * P], id_sb)
                nc.scalar.activation(
                    QT[:, i, :], pstr, FT.Identity, bias=bq_sb[:, i : i + 1]
                )

            # ---- qt[b, t] = Q @ Wk ; cb = Q . bk in a separate PSUM bank ----
            psq = psqp.tile([BLOC, T], F32, name="psq", tag="psq")
            cb_ps = pstrp.tile([BLOC, 1], F32, name="cb_ps", tag="pstr")
            for i in range(NCH):
                nc.tensor.matmul(
                    psq[:, 0:512],
                    QT[:, i, :],
                    wk_sb[i][:, 0:512],
                    start=(i == 0),
                    stop=(i == NCH - 1),
                )
                nc.tensor.matmul(
                    psq[:, 512:1024],
                    QT[:, i, :],
                    wk_sb[i][:, 512:1024],
                    start=(i == 0),
                    stop=(i == NCH - 1),
                )
                nc.tensor.matmul(
                    cb_ps,
                    QT[:, i, :],
                    bk_sb[:, i : i + 1],
                    start=(i == 0),
                    stop=(i == NCH - 1),
                )
            qt_sb = small.tile([BLOC, T + 1], F32, name="qt_sb", tag="qt_sb")
            nc.scalar.mul(qt_sb[:, 0:T], psq, SCALE)  # fold 1/sqrt(A) here
            nc.scalar.mul(qt_sb[:, T : T + 1], cb_ps, SCALE)

            # ---- broadcast qt rows to 128 partitions via DRAM round-trip ----
            qdram = dram.tile([BLOC, T + 1], F32, name="qdram", tag="qdram")
            nc.sync.dma_start(out=qdram, in_=qt_sb)
            qb_t = [
                qbp.tile([P, T + 1], F32, name=f"qbt{b}", tag=f"qbt{b}")
                for b in range(BLOC)
            ]
            for b in range(BLOC):
                nc.gpsimd.dma_start(out=qb_t[b], in_=qdram[b].partition_broadcast(P))

            # ---- main stream: e = (r . qt) + cb ; ww += e.T-weighted rows ----
            for b in range(BLOC):
                # ww accumulates as a [1, 1024] PSUM row (2 banks = 2 zero
                # regions; one start/stop pair per 512-wide half).
                ps_ww = pswwp.tile([1, T], F32, name="ps_ww", tag="ps_ww")
                for ti in range(NTI):
                    if b == 0 and ti < 2:
                        rt = rt_pre[ti]
                    else:
                        rt = rp.tile([P, LTILE, T], F32, name="rt", tag="rt")
                        nc.sync.dma_start(
                            out=rt,
                            in_=rev[
                                b, ti * ROWS_PER_TILE : (ti + 1) * ROWS_PER_TILE, :
                            ].rearrange("(p f) t -> p f t", f=LTILE),
                        )
                    e_raw = ep.tile([P, LTILE], F32, name="e_raw", tag="e_raw")
                    e_t = ep.tile([P, LTILE], F32, name="e_t", tag="e_t")
                    for i in range(LTILE):
                        # fused multiply + free-dim reduce on DVE:
                        # scr = r * qt_bcast ; e_raw = sum(scr)
                        scr = scrp.tile([P, T], F32, name="scr", tag="scr")
                        nc.vector.scalar_tensor_tensor(
                            out=scr,
                            in0=rt[:, i, :],
                            scalar=1.0,
                            in1=qb_t[b][:, 0:T],
                            op0=OP.bypass,
                            op1=OP.mult,
                            accum_out=e_raw[:, i : i + 1],
                        )
                        # e = e_raw + cb on ScalarE (cb pre-scaled by 1/sqrt(A))
                        nc.scalar.activation(
                            e_t[:, i : i + 1],
                            e_raw[:, i : i + 1],
                            FT.Identity,
                            bias=qb_t[b][:, T : T + 1],
                        )
                        # ww[0, :] += e_slice.T @ r_slice  (e stationary,
                        # reviewer tile streams as the moving operand)
                        for h in range(2):
                            nc.tensor.matmul(
                                ps_ww[0:1, h * 512 : (h + 1) * 512],
                                e_t[:, i : i + 1],
                                rt[:, i, h * 512 : (h + 1) * 512],
                                start=(ti == 0 and i == 0),
                                stop=(ti == NTI - 1 and i == LTILE - 1),
                            )
                ww_sb = wwp.tile([1, T], F32, name="ww_sb", tag="ww_sb")
                nc.scalar.copy(ww_sb, ps_ww)
                nc.sync.dma_start(out=out[b : b + 1, :], in_=ww_sb)

    nc.compile()
    return nc


_NC = None


def _get_nc():
    global _NC
    if _NC is None:
        _NC = _build()
    return _NC


def _in_maps(submitter_emb, reviewer_emb, Wq, bq, Wk, bk):
    submitter_emb = np.ascontiguousarray(submitter_emb, dtype=np.float32)
    reviewer_emb = np.ascontiguousarray(reviewer_emb, dtype=np.float32)
    # host-side layout prep (no arithmetic): transposes / chunking for DMA
    wqt_np = np.ascontiguousarray(np.asarray(Wq, dtype=np.float32).T)
    wk_np = np.ascontiguousarray(Wk, dtype=np.float32)
    bqc = np.ascontiguousarray(np.asarray(bq, dtype=np.float32).reshape(NCH, P).T)
    bkc = np.ascontiguousarray(np.asarray(bk, dtype=np.float32).reshape(NCH, P).T)
    ident = np.eye(BLOC, dtype=np.float32)

    in_maps = []
    for core in range(NCORES):
        lo, hi = core * BLOC, (core + 1) * BLOC
        # [BLOC, T] -> [128 p, NCH c, BLOC b] with t = 128*c + p
        subt = np.ascontiguousarray(
            submitter_emb[lo:hi].T.reshape(NCH, P, BLOC).transpose(1, 0, 2)
        )
        in_maps.append(
            {
                "subt": subt,
                "rev": reviewer_emb[lo:hi],
                "wqt": wqt_np,
                "bqc": bqc,
                "wk": wk_np,
                "bkc": bkc,
                "ident": ident,
            }
        )
    return in_maps


def kernel(
    submitter_emb: np.ndarray,
    reviewer_emb: np.ndarray,
    Wq: np.ndarray,
    bq: np.ndarray,
    Wk: np.ndarray,
    bk: np.ndarray,
) -> np.ndarray:
    nc = _get_nc()
    in_maps = _in_maps(submitter_emb, reviewer_emb, Wq, bq, Wk, bk)
    res = run_bass_kernel_spmd(nc, in_maps, core_ids=list(range(NCORES)))
    return np.concatenate([res.results[c]["out"] for c in range(NCORES)], axis=0)


# revision 18
# speedup vs baseline: 1.1386x; 1.1386x over previous
"""Trainium2 Bass kernel for nn_Attention_Module (submitter/reviewer attention pooling).

Reference math:
    Q  = submitter_emb @ Wq.T + bq                      [B, A]
    K  = einsum('blt,at->bla', reviewer_emb, Wk) + bk   [B, L, A]
    e  = einsum('ba,bla->bl', Q, K) / sqrt(A)           [B, L]
    ww = einsum('bl,blt->bt', e, reviewer_emb)          [B, T]

Algebraic rewrite used here (exact, just reassociation):
    qt[b, t] = sum_a Q[b, a] * Wk[a, t]        (= Q @ Wk,   [B, T], tiny)
    cb[b]    = sum_a Q[b, a] * bk[a]           (scalar per batch)
    e[b, l]  = (reviewer[b, l, :] . qt[b, :] + cb[b]) / sqrt(A)
    ww[b, t] = sum_l e[b, l] * reviewer[b, l, t]

This collapses the 137-GFLOP K matmul into a single streaming pass over
reviewer_emb: one fused DVE multiply+reduce (scalar_tensor_tensor with
accum_out) per tile for e, and PE matmuls with e as the stationary
operand (lhsT [128,1]) streaming the reviewer tile as the moving
operand, accumulating ww as a [1, 1024] PSUM row.

Sharding: data-parallel over batch B=32 -> 4 batches per core x 8 cores.
Weights replicated.  No cross-core communication; host concatenates.

Host-side prep is layout-only (transpose/reshape of inputs for DMA
efficiency); all input-dependent arithmetic runs on device in fp32.
"""

import numpy as np

import concourse.bass as bass
import concourse.bacc as bacc
import concourse.tile as tile
from concourse import mybir
from concourse.bass_utils import run_bass_kernel_spmd

# Problem shapes (hardcoded per contract)
B, L, T, A = 32, 2048, 1024, 1024
NCORES = 8
BLOC = B // NCORES          # 4 batches per core
P = 128                     # partitions
NCH = T // P                # 8 chunks of 128 along T/A
LTILE = 4                   # reviewer rows per partition per DMA tile
ROWS_PER_TILE = P * LTILE   # 512 rows -> 2 MiB per DMA
NTI = L // ROWS_PER_TILE    # 4 DMA tiles per batch
SCALE = 1.0 / float(np.sqrt(A))

F32 = mybir.dt.float32
FT = mybir.ActivationFunctionType
OP = mybir.AluOpType


def _build():
    nc = bacc.Bacc("TRN2", target_bir_lowering=False, debug=False, num_devices=NCORES)

    # subt: submitter slice pre-tiled to [128, NCH, BLOC] (t-major on partitions)
    subt = nc.dram_tensor("subt", [P, NCH, BLOC], F32, kind="ExternalInput").ap()
    rev = nc.dram_tensor("rev", [BLOC, L, T], F32, kind="ExternalInput").ap()
    # wqt: Wq.T, [T, A] row-major
    wqt = nc.dram_tensor("wqt", [T, A], F32, kind="ExternalInput").ap()
    # bqc/bkc: biases chunked [128, NCH] with element (p, c) = bias[128c + p]
    bqc = nc.dram_tensor("bqc", [P, NCH], F32, kind="ExternalInput").ap()
    wk = nc.dram_tensor("wk", [A, T], F32, kind="ExternalInput").ap()
    bkc = nc.dram_tensor("bkc", [P, NCH], F32, kind="ExternalInput").ap()
    ident = nc.dram_tensor("ident", [BLOC, BLOC], F32, kind="ExternalInput").ap()
    out = nc.dram_tensor("out", [BLOC, T], F32, kind="ExternalOutput").ap()

    with tile.TileContext(nc) as tc:
        with (
            tc.tile_pool(name="small", bufs=1) as small,
            tc.tile_pool(name="wqtp", bufs=1) as wqtp,
            tc.tile_pool(name="wkp", bufs=1) as wkp,
            tc.tile_pool(name="qb", bufs=1) as qbp,
            tc.tile_pool(name="rp", bufs=5) as rp,
            tc.tile_pool(name="scr", bufs=3) as scrp,
            tc.tile_pool(name="ep", bufs=4) as ep,
            tc.tile_pool(name="wwp", bufs=2) as wwp,
            tc.tile_pool(name="dram", bufs=1, space="DRAM") as dram,
            tc.tile_pool(name="psQ", bufs=1, space="PSUM") as psQp,
            tc.tile_pool(name="pstr", bufs=2, space="PSUM") as pstrp,
            tc.tile_pool(name="psq", bufs=1, space="PSUM") as psqp,
            tc.tile_pool(name="psww", bufs=1, space="PSUM") as pswwp,
        ):
            # ---- small loads (already laid out by host) ----
            subt_sb = small.tile([P, NCH, BLOC], F32, name="subt_sb", tag="subt_sb")
            nc.sync.dma_start(out=subt_sb, in_=subt)
            bq_sb = small.tile([P, NCH], F32, name="bq_sb", tag="bq_sb")
            nc.sync.dma_start(out=bq_sb, in_=bqc)
            bk_sb = small.tile([P, NCH], F32, name="bk_sb", tag="bk_sb")
            nc.sync.dma_start(out=bk_sb, in_=bkc)
            id_sb = small.tile([BLOC, BLOC], F32, name="id_sb", tag="id_sb")
            nc.sync.dma_start(out=id_sb, in_=ident)

            # ---- weight loads: wqT[j] = [128 t, 1024 a], wk[i] = [128 a, 1024 t] ----
            wqT = [
                wqtp.tile([P, A], F32, name=f"wqT{j}", tag=f"wqT{j}")
                for j in range(NCH)
            ]
            for j in range(NCH):
                nc.sync.dma_start(out=wqT[j], in_=wqt[j * P : (j + 1) * P, :])
            wk_sb = [
                wkp.tile([P, T], F32, name=f"wk{i}", tag=f"wk{i}") for i in range(NCH)
            ]
            for i in range(NCH):
                nc.sync.dma_start(out=wk_sb[i], in_=wk[i * P : (i + 1) * P, :])

            # ---- Q[b, a] = s @ Wq.T : Wq.T streams as the moving operand,
            #      so these matmuls overlap the weight-DMA wave chunk by chunk.
            psQ = psQp.tile([BLOC, A], F32, name="psQ", tag="psQ")
            for j in range(NCH):
                for h in range(2):
                    nc.tensor.matmul(
                        psQ[:, h * 512 : (h + 1) * 512],
                        subt_sb[:, j, :],
                        wqT[j][:, h * 512 : (h + 1) * 512],
                        start=(j == 0),
                        stop=(j == NCH - 1),
                    )
            Q_sb = small.tile([BLOC, A], F32, name="Q_sb", tag="Q_sb")
            nc.scalar.copy(Q_sb, psQ)

            # ---- QT chunks [128 a, BLOC] via tiny PE transposes; add bq here ----
            QT = small.tile([P, NCH, BLOC], F32, name="QT", tag="QT")
            for i in range(NCH):
                pstr = pstrp.tile([P, BLOC], F32, name="pstr", tag="pstr")
                nc.tensor.transpose(pstr, Q_sb[:, i * P : (i + 1) * P], id_sb)
                nc.scalar.activation(
                    QT[:, i, :], pstr, FT.Identity, bias=bq_sb[:, i : i + 1]
                )

            # ---- qt[b, t] = Q @ Wk on two concurrent PE col-groups;
            #      cb = Q . bk in a separate PSUM bank ----
            psq = psqp.tile([36, T], F32, name="psq", tag="psq")
            cb_ps = pstrp.tile([BLOC, 1], F32, name="cb_ps", tag="pstr")
            for i in range(NCH):
                nc.tensor.matmul(
                    psq[0:BLOC, 0:512],
                    QT[:, i, :],
                    wk_sb[i][:, 0:512],
                    start=(i == 0),
                    stop=(i == NCH - 1),
                    tile_position=(0, 0),
                )
                nc.tensor.matmul(
                    psq[32 : 32 + BLOC, 512:1024],
                    QT[:, i, :],
                    wk_sb[i][:, 512:1024],
                    start=(i == 0),
                    stop=(i == NCH - 1),
                    tile_position=(0, 32),
                )
                nc.tensor.matmul(
                    cb_ps,
                    QT[:, i, :],
                    bk_sb[:, i : i + 1],
                    start=(i == 0),
                    stop=(i == NCH - 1),
                )
            # fold 1/sqrt(A) here; halves sit on partitions 0-3 / 32-35 and
            # get re-joined by the DMA gather into qdram below
            qts0 = small.tile([BLOC, 512], F32, name="qts0", tag="qts0")
            nc.scalar.mul(qts0, psq[0:BLOC, 0:512], SCALE)
            qts1 = small.tile([36, 512], F32, name="qts1", tag="qts1")
            nc.scalar.mul(qts1[32 : 32 + BLOC, :], psq[32 : 32 + BLOC, 512:1024], SCALE)
            qtc = small.tile([BLOC, 1], F32, name="qtc", tag="qtc")
            nc.scalar.mul(qtc, cb_ps, SCALE)

            # ---- broadcast qt rows to 128 partitions via DRAM round-trip ----
            qdram = dram.tile([BLOC, T + 1], F32, name="qdram", tag="qdram")
            nc.sync.dma_start(out=qdram[:, 0:512], in_=qts0)
            nc.sync.dma_start(out=qdram[:, 512:1024], in_=qts1[32 : 32 + BLOC, :])
            nc.sync.dma_start(out=qdram[:, 1024 : T + 1], in_=qtc)
            qb_all = qbp.tile([P, BLOC, T + 1], F32, name="qb_all", tag="qb_all")
            nc.gpsimd.dma_start(
                out=qb_all.rearrange("p b t -> p (b t)"),
                in_=qdram.rearrange("b t -> (b t)").partition_broadcast(P),
            )
            qb_t = [qb_all[:, b, :] for b in range(BLOC)]

            # ---- main stream: e = (r . qt) + cb ; ww += e.T-weighted rows ----
            for b in range(BLOC):
                # ww halves accumulate on two concurrent PE col-groups:
                # half 0 -> psum row 0 cols 0:512 (bank 0), half 1 -> psum
                # row 32 cols 512:1024 (bank 1); one start/stop pair each.
                ps_ww = pswwp.tile([33, T], F32, name="ps_ww", tag="ps_ww")
                for ti in range(NTI):
                    rt = rp.tile([P, LTILE, T], F32, name="rt", tag="rt")
                    nc.sync.dma_start(
                        out=rt,
                        in_=rev[
                            b, ti * ROWS_PER_TILE : (ti + 1) * ROWS_PER_TILE, :
                        ].rearrange("(p f) t -> p f t", f=LTILE),
                    )
                    e_raw = ep.tile([P, LTILE], F32, name="e_raw", tag="e_raw")
                    e_t = ep.tile([P, LTILE], F32, name="e_t", tag="e_t")
                    for i in range(LTILE):
                        # fused multiply + free-dim reduce on DVE:
                        # scr = r * qt_bcast ; e_raw = sum(scr)
                        scr = scrp.tile([P, T], F32, name="scr", tag="scr")
                        nc.vector.scalar_tensor_tensor(
                            out=scr,
                            in0=rt[:, i, :],
                            scalar=1.0,
                            in1=qb_t[b][:, 0:T],
                            op0=OP.bypass,
                            op1=OP.mult,
                            accum_out=e_raw[:, i : i + 1],
                        )
                        # e = e_raw + cb on ScalarE (cb pre-scaled by 1/sqrt(A))
                        nc.scalar.activation(
                            e_t[:, i : i + 1],
                            e_raw[:, i : i + 1],
                            FT.Identity,
                            bias=qb_t[b][:, T : T + 1],
                        )
                        # ww[0, :] += e_slice.T @ r_slice  (e stationary,
                        # reviewer tile streams as the moving operand)
                        for h in range(2):
                            nc.tensor.matmul(
                                ps_ww[
                                    32 * h : 32 * h + 1, h * 512 : (h + 1) * 512
                                ],
                                e_t[:, i : i + 1],
                                rt[:, i, h * 512 : (h + 1) * 512],
                                start=(ti == 0 and i == 0),
                                stop=(ti == NTI - 1 and i == LTILE - 1),
                                tile_position=(0, 32 * h),
                            )
                ww_sb = wwp.tile([33, T], F32, name="ww_sb", tag="ww_sb")
                nc.scalar.copy(ww_sb[0:1, 0:512], ps_ww[0:1, 0:512])
                nc.scalar.copy(ww_sb[32:33, 512:1024], ps_ww[32:33, 512:1024])
                nc.sync.dma_start(out=out[b : b + 1, 0:512], in_=ww_sb[0:1, 0:512])
                nc.sync.dma_start(
                    out=out[b : b + 1, 512:1024], in_=ww_sb[32:33, 512:1024]
                )

    nc.compile()
    return nc


_NC = None


def _get_nc():
    global _NC
    if _NC is None:
        _NC = _build()
    return _NC


def _in_maps(submitter_emb, reviewer_emb, Wq, bq, Wk, bk):
    submitter_emb = np.ascontiguousarray(submitter_emb, dtype=np.float32)
    reviewer_emb = np.ascontiguousarray(reviewer_emb, dtype=np.float32)
    # host-side layout prep (no arithmetic): transposes / chunking for DMA
    wqt_np = np.ascontiguousarray(np.asarray(Wq, dtype=np.float32).T)
    wk_np = np.ascontiguousarray(Wk, dtype=np.float32)
    bqc = np.ascontiguousarray(np.asarray(bq, dtype=np.float32).reshape(NCH, P).T)
    bkc = np.ascontiguousarray(np.asarray(bk, dtype=np.float32).reshape(NCH, P).T)
    ident = np.eye(BLOC, dtype=np.float32)

    in_maps = []
    for core in range(NCORES):
        lo, hi = core * BLOC, (core + 1) * BLOC
        # [BLOC, T] -> [128 p, NCH c, BLOC b] with t = 128*c + p
        subt = np.ascontiguousarray(
            submitter_emb[lo:hi].T.reshape(NCH, P, BLOC).transpose(1, 0, 2)
        )
        in_maps.append(
            {
                "subt": subt,
                "rev": reviewer_emb[lo:hi],
                "wqt": wqt_np,
                "bqc": bqc,
                "wk": wk_np,
                "bkc": bkc,
                "ident": ident,
            }
        )
    return in_maps


def kernel(
    submitter_emb: np.ndarray,
    reviewer_emb: np.ndarray,
    Wq: np.ndarray,
    bq: np.ndarray,
    Wk: np.ndarray,
    bk: np.ndarray,
) -> np.ndarray:
    nc = _get_nc()
    in_maps = _in_maps(submitter_emb, reviewer_emb, Wq, bq, Wk, bk)
    res = run_bass_kernel_spmd(nc, in_maps, core_ids=list(range(NCORES)))
    return np.concatenate([res.results[c]["out"] for c in range(NCORES)], axis=0)


# revision 19
# speedup vs baseline: 1.1784x; 1.0349x over previous
"""Trainium2 Bass kernel for nn_Attention_Module (submitter/reviewer attention pooling).

Reference math:
    Q  = submitter_emb @ Wq.T + bq                      [B, A]
    K  = einsum('blt,at->bla', reviewer_emb, Wk) + bk   [B, L, A]
    e  = einsum('ba,bla->bl', Q, K) / sqrt(A)           [B, L]
    ww = einsum('bl,blt->bt', e, reviewer_emb)          [B, T]

Algebraic rewrite used here (exact, just reassociation):
    qt[b, t] = sum_a Q[b, a] * Wk[a, t]        (= Q @ Wk,   [B, T], tiny)
    cb[b]    = sum_a Q[b, a] * bk[a]           (scalar per batch)
    e[b, l]  = (reviewer[b, l, :] . qt[b, :] + cb[b]) / sqrt(A)
    ww[b, t] = sum_l e[b, l] * reviewer[b, l, t]

This collapses the 137-GFLOP K matmul into a single streaming pass over
reviewer_emb: one fused DVE multiply+reduce (scalar_tensor_tensor with
accum_out) per tile for e, and PE matmuls with e as the stationary
operand (lhsT [128,1]) streaming the reviewer tile as the moving
operand, accumulating ww as a [1, 1024] PSUM row.

Sharding: data-parallel over batch B=32 -> 4 batches per core x 8 cores.
Weights replicated.  No cross-core communication; host concatenates.

Host-side prep is layout-only (transpose/reshape of inputs for DMA
efficiency); all input-dependent arithmetic runs on device in fp32.
"""

import numpy as np

import concourse.bass as bass
import concourse.bacc as bacc
import concourse.tile as tile
from concourse.tile_rust import add_dep_helper
from concourse import mybir
from concourse.bass_utils import run_bass_kernel_spmd

# Problem shapes (hardcoded per contract)
B, L, T, A = 32, 2048, 1024, 1024
NCORES = 8
BLOC = B // NCORES          # 4 batches per core
P = 128                     # partitions
NCH = T // P                # 8 chunks of 128 along T/A
LTILE = 4                   # reviewer rows per partition per DMA tile
ROWS_PER_TILE = P * LTILE   # 512 rows -> 2 MiB per DMA
NTI = L // ROWS_PER_TILE    # 4 DMA tiles per batch
SCALE = 1.0 / float(np.sqrt(A))

F32 = mybir.dt.float32
FT = mybir.ActivationFunctionType
OP = mybir.AluOpType


def _build():
    nc = bacc.Bacc("TRN2", target_bir_lowering=False, debug=False, num_devices=NCORES)

    # subt: submitter slice pre-tiled to [128, NCH, BLOC] (t-major on partitions)
    subt = nc.dram_tensor("subt", [P, NCH, BLOC], F32, kind="ExternalInput").ap()
    rev = nc.dram_tensor("rev", [BLOC, L, T], F32, kind="ExternalInput").ap()
    # wqt: Wq.T, [T, A] row-major
    wqt = nc.dram_tensor("wqt", [T, A], F32, kind="ExternalInput").ap()
    # bqc/bkc: biases chunked [128, NCH] with element (p, c) = bias[128c + p]
    bqc = nc.dram_tensor("bqc", [P, NCH], F32, kind="ExternalInput").ap()
    wk = nc.dram_tensor("wk", [A, T], F32, kind="ExternalInput").ap()
    bkc = nc.dram_tensor("bkc", [P, NCH], F32, kind="ExternalInput").ap()
    ident = nc.dram_tensor("ident", [BLOC, BLOC], F32, kind="ExternalInput").ap()
    out = nc.dram_tensor("out", [BLOC, T], F32, kind="ExternalOutput").ap()

    with tile.TileContext(nc) as tc:
        with (
            tc.tile_pool(name="small", bufs=1) as small,
            tc.tile_pool(name="wqtp", bufs=1) as wqtp,
            tc.tile_pool(name="wkp", bufs=1) as wkp,
            tc.tile_pool(name="qb", bufs=1) as qbp,
            tc.tile_pool(name="rp", bufs=5) as rp,
            tc.tile_pool(name="scr", bufs=3) as scrp,
            tc.tile_pool(name="ep", bufs=4) as ep,
            tc.tile_pool(name="wwp", bufs=2) as wwp,
            tc.tile_pool(name="dram", bufs=1, space="DRAM") as dram,
            tc.tile_pool(name="psQ", bufs=1, space="PSUM") as psQp,
            tc.tile_pool(name="pstr", bufs=2, space="PSUM") as pstrp,
            tc.tile_pool(name="psq", bufs=1, space="PSUM") as psqp,
            tc.tile_pool(name="psww", bufs=1, space="PSUM") as pswwp,
        ):
            # ---- small loads (already laid out by host) ----
            subt_sb = small.tile([P, NCH, BLOC], F32, name="subt_sb", tag="subt_sb")
            nc.sync.dma_start(out=subt_sb, in_=subt)
            bq_sb = small.tile([P, NCH], F32, name="bq_sb", tag="bq_sb")
            nc.sync.dma_start(out=bq_sb, in_=bqc)
            bk_sb = small.tile([P, NCH], F32, name="bk_sb", tag="bk_sb")
            nc.sync.dma_start(out=bk_sb, in_=bkc)
            id_sb = small.tile([BLOC, BLOC], F32, name="id_sb", tag="id_sb")
            nc.sync.dma_start(out=id_sb, in_=ident)

            # ---- weight loads: wqT[j] = [128 t, 1024 a], wk[i] = [128 a, 1024 t] ----
            wqT = [
                wqtp.tile([P, A], F32, name=f"wqT{j}", tag=f"wqT{j}")
                for j in range(NCH)
            ]
            for j in range(NCH):
                nc.sync.dma_start(out=wqT[j], in_=wqt[j * P : (j + 1) * P, :])
            wk_sb = [
                wkp.tile([P, T], F32, name=f"wk{i}", tag=f"wk{i}") for i in range(NCH)
            ]
            for i in range(NCH):
                nc.sync.dma_start(out=wk_sb[i], in_=wk[i * P : (i + 1) * P, :])

            # ---- Q[b, a] = s @ Wq.T : Wq.T streams as the moving operand,
            #      so these matmuls overlap the weight-DMA wave chunk by chunk.
            psQ = psQp.tile([BLOC, A], F32, name="psQ", tag="psQ")
            for j in range(NCH):
                for h in range(2):
                    nc.tensor.matmul(
                        psQ[:, h * 512 : (h + 1) * 512],
                        subt_sb[:, j, :],
                        wqT[j][:, h * 512 : (h + 1) * 512],
                        start=(j == 0),
                        stop=(j == NCH - 1),
                    )
            Q_sb = small.tile([BLOC, A], F32, name="Q_sb", tag="Q_sb")
            nc.scalar.copy(Q_sb, psQ)

            # ---- QT chunks [128 a, BLOC] via tiny PE transposes; add bq here ----
            QT = small.tile([P, NCH, BLOC], F32, name="QT", tag="QT")
            for i in range(NCH):
                pstr = pstrp.tile([P, BLOC], F32, name="pstr", tag="pstr")
                nc.tensor.transpose(pstr, Q_sb[:, i * P : (i + 1) * P], id_sb)
                nc.scalar.activation(
                    QT[:, i, :], pstr, FT.Identity, bias=bq_sb[:, i : i + 1]
                )

            # ---- qt[b, t] = Q @ Wk on two concurrent PE col-groups;
            #      cb = Q . bk in a separate PSUM bank ----
            psq = psqp.tile([36, T], F32, name="psq", tag="psq")
            cb_ps = pstrp.tile([BLOC, 1], F32, name="cb_ps", tag="pstr")
            for i in range(NCH):
                nc.tensor.matmul(
                    psq[0:BLOC, 0:512],
                    QT[:, i, :],
                    wk_sb[i][:, 0:512],
                    start=(i == 0),
                    stop=(i == NCH - 1),
                    tile_position=(0, 0),
                )
                nc.tensor.matmul(
                    psq[32 : 32 + BLOC, 512:1024],
                    QT[:, i, :],
                    wk_sb[i][:, 512:1024],
                    start=(i == 0),
                    stop=(i == NCH - 1),
                    tile_position=(0, 32),
                )
                nc.tensor.matmul(
                    cb_ps,
                    QT[:, i, :],
                    bk_sb[:, i : i + 1],
                    start=(i == 0),
                    stop=(i == NCH - 1),
                )
            # fold 1/sqrt(A) here; halves sit on partitions 0-3 / 32-35 and
            # get re-joined by the DMA gather into qdram below
            qts0 = small.tile([BLOC, 512], F32, name="qts0", tag="qts0")
            nc.scalar.mul(qts0, psq[0:BLOC, 0:512], SCALE)
            qts1 = small.tile([36, 512], F32, name="qts1", tag="qts1")
            nc.scalar.mul(qts1[32 : 32 + BLOC, :], psq[32 : 32 + BLOC, 512:1024], SCALE)
            qtc = small.tile([BLOC, 1], F32, name="qtc", tag="qtc")
            nc.scalar.mul(qtc, cb_ps, SCALE)

            # ---- broadcast qt rows to 128 partitions via DRAM round-trip ----
            qdram = dram.tile([BLOC, T + 1], F32, name="qdram", tag="qdram")
            nc.sync.dma_start(out=qdram[:, 0:512], in_=qts0)
            nc.sync.dma_start(out=qdram[:, 512:1024], in_=qts1[32 : 32 + BLOC, :])
            nc.sync.dma_start(out=qdram[:, 1024 : T + 1], in_=qtc)
            qb_all = qbp.tile([P, BLOC, T + 1], F32, name="qb_all", tag="qb_all")
            nc.gpsimd.dma_start(
                out=qb_all.rearrange("p b t -> p (b t)"),
                in_=qdram.rearrange("b t -> (b t)").partition_broadcast(P),
            )
            qb_t = [qb_all[:, b, :] for b in range(BLOC)]

            # ---- main stream: e = (r . qt) + cb ; ww += e.T-weighted rows ----
            rt_dmas = []  # chain reviewer DMAs depth-3 so they complete in
            # order (unchained, the round-robin queues finish the whole
            # first wave together, gating the first compute tile)
            for b in range(BLOC):
                # ww halves accumulate on two concurrent PE col-groups:
                # half 0 -> psum row 0 cols 0:512 (bank 0), half 1 -> psum
                # row 32 cols 512:1024 (bank 1); one start/stop pair each.
                ps_ww = pswwp.tile([33, T], F32, name="ps_ww", tag="ps_ww")
                for ti in range(NTI):
                    rt = rp.tile([P, LTILE, T], F32, name="rt", tag="rt")
                    d = nc.sync.dma_start(
                        out=rt,
                        in_=rev[
                            b, ti * ROWS_PER_TILE : (ti + 1) * ROWS_PER_TILE, :
                        ].rearrange("(p f) t -> p f t", f=LTILE),
                    )
                    if len(rt_dmas) >= 3:
                        add_dep_helper(
                            d.ins, rt_dmas[-3].ins, reason="stagger rt stream"
                        )
                    rt_dmas.append(d)
                    e_raw = ep.tile([P, LTILE], F32, name="e_raw", tag="e_raw")
                    e_t = ep.tile([P, LTILE], F32, name="e_t", tag="e_t")
                    for i in range(LTILE):
                        # fused multiply + free-dim reduce on DVE:
                        # scr = r * qt_bcast ; e_raw = sum(scr)
                        scr = scrp.tile([P, T], F32, name="scr", tag="scr")
                        nc.vector.scalar_tensor_tensor(
                            out=scr,
                            in0=rt[:, i, :],
                            scalar=1.0,
                            in1=qb_t[b][:, 0:T],
                            op0=OP.bypass,
                            op1=OP.mult,
                            accum_out=e_raw[:, i : i + 1],
                        )
                        # e = e_raw + cb on ScalarE (cb pre-scaled by 1/sqrt(A))
                        nc.scalar.activation(
                            e_t[:, i : i + 1],
                            e_raw[:, i : i + 1],
                            FT.Identity,
                            bias=qb_t[b][:, T : T + 1],
                        )
                        # ww[0, :] += e_slice.T @ r_slice  (e stationary,
                        # reviewer tile streams as the moving operand)
                        for h in range(2):
                            nc.tensor.matmul(
                                ps_ww[
                                    32 * h : 32 * h + 1, h * 512 : (h + 1) * 512
                                ],
                                e_t[:, i : i + 1],
                                rt[:, i, h * 512 : (h + 1) * 512],
                                start=(ti == 0 and i == 0),
                                stop=(ti == NTI - 1 and i == LTILE - 1),
                                tile_position=(0, 32 * h),
                            )
                ww_sb = wwp.tile([33, T], F32, name="ww_sb", tag="ww_sb")
                nc.scalar.copy(ww_sb[0:1, 0:512], ps_ww[0:1, 0:512])
                nc.scalar.copy(ww_sb[32:33, 512:1024], ps_ww[32:33, 512:1024])
                nc.sync.dma_start(out=out[b : b + 1, 0:512], in_=ww_sb[0:1, 0:512])
                nc.sync.dma_start(
                    out=out[b : b + 1, 512:1024], in_=ww_sb[32:33, 512:1024]
                )

    nc.compile()
    return nc


_NC = None


def _get_nc():
    global _NC
    if _NC is None:
        _NC = _build()
    return _NC


def _in_maps(submitter_emb, reviewer_emb, Wq, bq, Wk, bk):
    submitter_emb = np.ascontiguousarray(submitter_emb, dtype=np.float32)
    reviewer_emb = np.ascontiguousarray(reviewer_emb, dtype=np.float32)
    # host-side layout prep (no arithmetic): transposes / chunking for DMA
    wqt_np = np.ascontiguousarray(np.asarray(Wq, dtype=np.float32).T)
    wk_np = np.ascontiguousarray(Wk, dtype=np.float32)
    bqc = np.ascontiguousarray(np.asarray(bq, dtype=np.float32).reshape(NCH, P).T)
    bkc = np.ascontiguousarray(np.asarray(bk, dtype=np.float32).reshape(NCH, P).T)
    ident = np.eye(BLOC, dtype=np.float32)

    in_maps = []
    for core in range(NCORES):
        lo, hi = core * BLOC, (core + 1) * BLOC
        # [BLOC, T] -> [128 p, NCH c, BLOC b] with t = 128*c + p
        subt = np.ascontiguousarray(
            submitter_emb[lo:hi].T.reshape(NCH, P, BLOC).transpose(1, 0, 2)
        )
        in_maps.append(
            {
                "subt": subt,
                "rev": reviewer_emb[lo:hi],
                "wqt": wqt_np,
                "bqc": bqc,
                "wk": wk_np,
                "bkc": bkc,
                "ident": ident,
            }
        )
    return in_maps


def kernel(
    submitter_emb: np.ndarray,
    reviewer_emb: np.ndarray,
    Wq: np.ndarray,
    bq: np.ndarray,
    Wk: np.ndarray,
    bk: np.ndarray,
) -> np.ndarray:
    nc = _get_nc()
    in_maps = _in_maps(submitter_emb, reviewer_emb, Wq, bq, Wk, bk)
    res = run_bass_kernel_spmd(nc, in_maps, core_ids=list(range(NCORES)))
    return np.concatenate([res.results[c]["out"] for c in range(NCORES)], axis=0)


# revision 20
# speedup vs baseline: 1.3582x; 1.1526x over previous
"""Trainium2 Bass kernel for nn_Attention_Module (submitter/reviewer attention pooling).

Reference math:
    Q  = submitter_emb @ Wq.T + bq                      [B, A]
    K  = einsum('blt,at->bla', reviewer_emb, Wk) + bk   [B, L, A]
    e  = einsum('ba,bla->bl', Q, K) / sqrt(A)           [B, L]
    ww = einsum('bl,blt->bt', e, reviewer_emb)          [B, T]

Algebraic rewrite used here (exact, just reassociation):
    qt[b, t] = sum_a Q[b, a] * Wk[a, t]        (= Q @ Wk,   [B, T], tiny)
    cb[b]    = sum_a Q[b, a] * bk[a]           (scalar per batch)
    e[b, l]  = (reviewer[b, l, :] . qt[b, :] + cb[b]) / sqrt(A)
    ww[b, t] = sum_l e[b, l] * reviewer[b, l, t]

This collapses the 137-GFLOP K matmul into a single streaming pass over
reviewer_emb: one fused DVE multiply+reduce (scalar_tensor_tensor with
accum_out) per tile for e, and PE matmuls with e as the stationary
operand (lhsT [128,1]) streaming the reviewer tile as the moving
operand, accumulating ww as a [1, 1024] PSUM row.

Sharding: data-parallel over batch B=32 -> 4 batches per core x 8 cores.
Weights replicated.  No cross-core communication; host concatenates.

Host-side prep is layout-only (transpose/reshape of inputs for DMA
efficiency); all input-dependent arithmetic runs on device in fp32.
"""

import numpy as np

import concourse.bass as bass
import concourse.bacc as bacc
import concourse.tile as tile
from concourse.tile_rust import add_dep_helper
from concourse import mybir
from concourse.bass_utils import run_bass_kernel_spmd

# Problem shapes (hardcoded per contract)
B, L, T, A = 32, 2048, 1024, 1024
NCORES = 8
BLOC = B // NCORES          # 4 batches per core
P = 128                     # partitions
NCH = T // P                # 8 chunks of 128 along T/A
LTILE = 4                   # reviewer rows per partition per DMA tile
ROWS_PER_TILE = P * LTILE   # 512 rows -> 2 MiB per DMA
NTI = L // ROWS_PER_TILE    # 4 DMA tiles per batch
SCALE = 1.0 / float(np.sqrt(A))

F32 = mybir.dt.float32
FT = mybir.ActivationFunctionType
OP = mybir.AluOpType


def _build():
    nc = bacc.Bacc("TRN2", target_bir_lowering=False, debug=False, num_devices=NCORES)

    # subt: submitter slice pre-tiled to [128, NCH, BLOC] (t-major on partitions)
    subt = nc.dram_tensor("subt", [P, NCH, BLOC], F32, kind="ExternalInput").ap()
    rev = nc.dram_tensor("rev", [BLOC, L, T], F32, kind="ExternalInput").ap()
    # wqt: Wq.T, [T, A] row-major
    wqt = nc.dram_tensor("wqt", [T, A], F32, kind="ExternalInput").ap()
    # bqc/bkc: biases chunked [128, NCH] with element (p, c) = bias[128c + p]
    bqc = nc.dram_tensor("bqc", [P, NCH], F32, kind="ExternalInput").ap()
    wk = nc.dram_tensor("wk", [A, T], F32, kind="ExternalInput").ap()
    bkc = nc.dram_tensor("bkc", [P, NCH], F32, kind="ExternalInput").ap()
    ident = nc.dram_tensor("ident", [BLOC, BLOC], F32, kind="ExternalInput").ap()
    out = nc.dram_tensor("out", [BLOC, T], F32, kind="ExternalOutput").ap()

    with tile.TileContext(nc) as tc:
        with (
            tc.tile_pool(name="small", bufs=1) as small,
            tc.tile_pool(name="wqtp", bufs=1) as wqtp,
            tc.tile_pool(name="wkp", bufs=1) as wkp,
            tc.tile_pool(name="qb", bufs=1) as qbp,
            tc.tile_pool(name="rp", bufs=5) as rp,
            tc.tile_pool(name="scr", bufs=3) as scrp,
            tc.tile_pool(name="ep", bufs=4) as ep,
            tc.tile_pool(name="wwp", bufs=2) as wwp,
            tc.tile_pool(name="dram", bufs=1, space="DRAM") as dram,
            tc.tile_pool(name="pstr", bufs=2, space="PSUM") as pstrp,
            tc.tile_pool(name="psq", bufs=1, space="PSUM") as psqp,
            tc.tile_pool(name="psww", bufs=2, space="PSUM") as pswwp,
        ):
            # ---- small loads (already laid out by host) ----
            subt_sb = small.tile([P, NCH, BLOC], F32, name="subt_sb", tag="subt_sb")
            nc.sync.dma_start(out=subt_sb, in_=subt)
            bq_sb = small.tile([P, NCH], F32, name="bq_sb", tag="bq_sb")
            nc.sync.dma_start(out=bq_sb, in_=bqc)
            bk_sb = small.tile([P, NCH], F32, name="bk_sb", tag="bk_sb")
            nc.sync.dma_start(out=bk_sb, in_=bkc)
            id_sb = small.tile([BLOC, BLOC], F32, name="id_sb", tag="id_sb")
            nc.sync.dma_start(out=id_sb, in_=ident)

            # ---- weight loads: wqT[j] = [128 t, 1024 a], wk[i] = [128 a, 1024 t] ----
            wqT = [
                wqtp.tile([P, A], F32, name=f"wqT{j}", tag=f"wqT{j}")
                for j in range(NCH)
            ]
            for j in range(NCH):
                nc.sync.dma_start(out=wqT[j], in_=wqt[j * P : (j + 1) * P, :])
            wk_sb = [
                wkp.tile([P, T], F32, name=f"wk{i}", tag=f"wk{i}") for i in range(NCH)
            ]
            for i in range(NCH):
                nc.sync.dma_start(out=wk_sb[i], in_=wk[i * P : (i + 1) * P, :])

            # ---- Q[b, a] = s @ Wq.T : Wq.T streams as the moving operand,
            #      so these matmuls overlap the weight-DMA wave chunk by chunk.
            psQ = psqp.tile([36, A], F32, name="psQ", tag="psq")[0:BLOC, :]
            for j in range(NCH):
                for h in range(2):
                    nc.tensor.matmul(
                        psQ[:, h * 512 : (h + 1) * 512],
                        subt_sb[:, j, :],
                        wqT[j][:, h * 512 : (h + 1) * 512],
                        start=(j == 0),
                        stop=(j == NCH - 1),
                    )
            Q_sb = small.tile([BLOC, A], F32, name="Q_sb", tag="Q_sb")
            nc.scalar.copy(Q_sb, psQ)

            # ---- QT chunks [128 a, BLOC] via tiny PE transposes; add bq here ----
            QT = small.tile([P, NCH, BLOC], F32, name="QT", tag="QT")
            for i in range(NCH):
                pstr = pstrp.tile([P, BLOC], F32, name="pstr", tag="pstr")
                nc.tensor.transpose(pstr, Q_sb[:, i * P : (i + 1) * P], id_sb)
                nc.scalar.activation(
                    QT[:, i, :], pstr, FT.Identity, bias=bq_sb[:, i : i + 1]
                )

            # ---- qt[b, t] = Q @ Wk on two concurrent PE col-groups;
            #      cb = Q . bk in a separate PSUM bank ----
            psq = psqp.tile([36, T], F32, name="psq", tag="psq")
            cb_ps = pstrp.tile([BLOC, 1], F32, name="cb_ps", tag="pstr")
            for i in range(NCH):
                nc.tensor.matmul(
                    psq[0:BLOC, 0:512],
                    QT[:, i, :],
                    wk_sb[i][:, 0:512],
                    start=(i == 0),
                    stop=(i == NCH - 1),
                    tile_position=(0, 0),
                )
                nc.tensor.matmul(
                    psq[32 : 32 + BLOC, 512:1024],
                    QT[:, i, :],
                    wk_sb[i][:, 512:1024],
                    start=(i == 0),
                    stop=(i == NCH - 1),
                    tile_position=(0, 32),
                )
                nc.tensor.matmul(
                    cb_ps,
                    QT[:, i, :],
                    bk_sb[:, i : i + 1],
                    start=(i == 0),
                    stop=(i == NCH - 1),
                )
            # fold 1/sqrt(A) here; halves sit on partitions 0-3 / 32-35 and
            # get re-joined by the DMA gather into qdram below
            qts0 = small.tile([BLOC, 512], F32, name="qts0", tag="qts0")
            nc.scalar.mul(qts0, psq[0:BLOC, 0:512], SCALE)
            qts1 = small.tile([36, 512], F32, name="qts1", tag="qts1")
            nc.scalar.mul(qts1[32 : 32 + BLOC, :], psq[32 : 32 + BLOC, 512:1024], SCALE)
            qtc = small.tile([BLOC, 1], F32, name="qtc", tag="qtc")
            nc.scalar.mul(qtc, cb_ps, SCALE)

            # ---- broadcast qt rows to 128 partitions via DRAM round-trip ----
            qdram = dram.tile([BLOC, T + 1], F32, name="qdram", tag="qdram")
            nc.gpsimd.dma_start(out=qdram[:, 0:512], in_=qts0)
            nc.gpsimd.dma_start(out=qdram[:, 512:1024], in_=qts1[32 : 32 + BLOC, :])
            nc.gpsimd.dma_start(out=qdram[:, 1024 : T + 1], in_=qtc)
            qb_all = qbp.tile([P, BLOC, T + 1], F32, name="qb_all", tag="qb_all")
            nc.gpsimd.dma_start(
                out=qb_all.rearrange("p b t -> p (b t)"),
                in_=qdram.rearrange("b t -> (b t)").partition_broadcast(P),
            )
            qb_t = [qb_all[:, b, :] for b in range(BLOC)]

            # ---- main stream: e = (r . qt) + cb ; ww += e.T-weighted rows ----
            rt_dmas = []  # chain reviewer DMAs depth-3 so they complete in
            # order (unchained, the round-robin queues finish the whole
            # first wave together, gating the first compute tile)
            for b in range(BLOC):
                # ww halves accumulate on two concurrent PE col-groups:
                # half 0 -> psum row 0 cols 0:512 (bank 0), half 1 -> psum
                # row 32 cols 512:1024 (bank 1); one start/stop pair each.
                ps_ww = pswwp.tile([33, T], F32, name="ps_ww", tag="ps_ww")
                for ti in range(NTI):
                    rt = rp.tile([P, LTILE, T], F32, name="rt", tag="rt")
                    d = nc.sync.dma_start(
                        out=rt,
                        in_=rev[
                            b, ti * ROWS_PER_TILE : (ti + 1) * ROWS_PER_TILE, :
                        ].rearrange("(p f) t -> p f t", f=LTILE),
                    )
                    if len(rt_dmas) >= 3:
                        add_dep_helper(
                            d.ins, rt_dmas[-3].ins, reason="stagger rt stream"
                        )
                    rt_dmas.append(d)
                    e_raw = ep.tile([P, LTILE], F32, name="e_raw", tag="e_raw")
                    e_t = ep.tile([P, LTILE], F32, name="e_t", tag="e_t")
                    for i in range(LTILE):
                        # fused multiply + free-dim reduce on DVE:
                        # scr = r * qt_bcast ; e_raw = sum(scr)
                        scr = scrp.tile([P, T], F32, name="scr", tag="scr")
                        nc.vector.scalar_tensor_tensor(
                            out=scr,
                            in0=rt[:, i, :],
                            scalar=1.0,
                            in1=qb_t[b][:, 0:T],
                            op0=OP.bypass,
                            op1=OP.mult,
                            accum_out=e_raw[:, i : i + 1],
                        )
                        # e = e_raw + cb on ScalarE (cb pre-scaled by 1/sqrt(A))
                        nc.scalar.activation(
                            e_t[:, i : i + 1],
                            e_raw[:, i : i + 1],
                            FT.Identity,
                            bias=qb_t[b][:, T : T + 1],
                        )
                        # ww[0, :] += e_slice.T @ r_slice  (e stationary,
                        # reviewer tile streams as the moving operand)
                        for h in range(2):
                            nc.tensor.matmul(
                                ps_ww[
                                    32 * h : 32 * h + 1, h * 512 : (h + 1) * 512
                                ],
                                e_t[:, i : i + 1],
                                rt[:, i, h * 512 : (h + 1) * 512],
                                start=(ti == 0 and i == 0),
                                stop=(ti == NTI - 1 and i == LTILE - 1),
                                tile_position=(0, 32 * h),
                            )
                ww_sb = wwp.tile([33, T], F32, name="ww_sb", tag="ww_sb")
                nc.scalar.copy(ww_sb[0:1, 0:512], ps_ww[0:1, 0:512])
                nc.scalar.copy(ww_sb[32:33, 512:1024], ps_ww[32:33, 512:1024])
                nc.scalar.dma_start(out=out[b : b + 1, 0:512], in_=ww_sb[0:1, 0:512])
                nc.scalar.dma_start(
                    out=out[b : b + 1, 512:1024], in_=ww_sb[32:33, 512:1024]
                )

    nc.compile()
    return nc


_NC = None


def _get_nc():
    global _NC
    if _NC is None:
        _NC = _build()
    return _NC


def _in_maps(submitter_emb, reviewer_emb, Wq, bq, Wk, bk):
    submitter_emb = np.ascontiguousarray(submitter_emb, dtype=np.float32)
    reviewer_emb = np.ascontiguousarray(reviewer_emb, dtype=np.float32)
    # host-side layout prep (no arithmetic): transposes / chunking for DMA
    wqt_np = np.ascontiguousarray(np.asarray(Wq, dtype=np.float32).T)
    wk_np = np.ascontiguousarray(Wk, dtype=np.float32)
    bqc = np.ascontiguousarray(np.asarray(bq, dtype=np.float32).reshape(NCH, P).T)
    bkc = np.ascontiguousarray(np.asarray(bk, dtype=np.float32).reshape(NCH, P).T)
    ident = np.eye(BLOC, dtype=np.float32)

    in_maps = []
    for core in range(NCORES):
        lo, hi = core * BLOC, (core + 1) * BLOC
        # [BLOC, T] -> [128 p, NCH c, BLOC b] with t = 128*c + p
        subt = np.ascontiguousarray(
            submitter_emb[lo:hi].T.reshape(NCH, P, BLOC).transpose(1, 0, 2)
        )
        in_maps.append(
            {
                "subt": subt,
                "rev": reviewer_emb[lo:hi],
                "wqt": wqt_np,
                "bqc": bqc,
                "wk": wk_np,
                "bkc": bkc,
                "ident": ident,
            }
        )
    return in_maps


def kernel(
    submitter_emb: np.ndarray,
    reviewer_emb: np.ndarray,
    Wq: np.ndarray,
    bq: np.ndarray,
    Wk: np.ndarray,
    bk: np.ndarray,
) -> np.ndarray:
    nc = _get_nc()
    in_maps = _in_maps(submitter_emb, reviewer_emb, Wq, bq, Wk, bk)
    res = run_bass_kernel_spmd(nc, in_maps, core_ids=list(range(NCORES)))
    return np.concatenate([res.results[c]["out"] for c in range(NCORES)], axis=0)


# revision 22
# speedup vs baseline: 1.4293x; 1.0524x over previous
"""Trainium2 Bass kernel for nn_Attention_Module (submitter/reviewer attention pooling).

Reference math:
    Q  = submitter_emb @ Wq.T + bq                      [B, A]
    K  = einsum('blt,at->bla', reviewer_emb, Wk) + bk   [B, L, A]
    e  = einsum('ba,bla->bl', Q, K) / sqrt(A)           [B, L]
    ww = einsum('bl,blt->bt', e, reviewer_emb)          [B, T]

Algebraic rewrite used here (exact, just reassociation):
    qt[b, t] = sum_a Q[b, a] * Wk[a, t]        (= Q @ Wk,   [B, T], tiny)
    cb[b]    = sum_a Q[b, a] * bk[a]           (scalar per batch)
    e[b, l]  = (reviewer[b, l, :] . qt[b, :] + cb[b]) / sqrt(A)
    ww[b, t] = sum_l e[b, l] * reviewer[b, l, t]

This collapses the 137-GFLOP K matmul into a single streaming pass over
reviewer_emb: one fused DVE multiply+reduce (scalar_tensor_tensor with
accum_out) per tile for e, and PE matmuls with e as the stationary
operand (lhsT [128,1]) streaming the reviewer tile as the moving
operand, accumulating ww as a [1, 1024] PSUM row.

Sharding: data-parallel over batch B=32 -> 4 batches per core x 8 cores.
Weights replicated.  No cross-core communication; host concatenates.

Host-side prep is layout-only (transpose/reshape of inputs for DMA
efficiency); all input-dependent arithmetic runs on device in fp32.
"""

import numpy as np

import concourse.bass as bass
import concourse.bacc as bacc
import concourse.tile as tile
from concourse.tile_rust import add_dep_helper
from concourse import mybir
from concourse.bass_utils import run_bass_kernel_spmd

# Problem shapes (hardcoded per contract)
B, L, T, A = 32, 2048, 1024, 1024
NCORES = 8
BLOC = B // NCORES          # 4 batches per core
P = 128                     # partitions
NCH = T // P                # 8 chunks of 128 along T/A
LTILE = 4                   # reviewer rows per partition per DMA tile
ROWS_PER_TILE = P * LTILE   # 512 rows -> 2 MiB per DMA
NTI = L // ROWS_PER_TILE    # 4 DMA tiles per batch
SCALE = 1.0 / float(np.sqrt(A))

F32 = mybir.dt.float32
FT = mybir.ActivationFunctionType
OP = mybir.AluOpType


def _build():
    nc = bacc.Bacc("TRN2", target_bir_lowering=False, debug=False, num_devices=NCORES)

    # subt: submitter slice pre-tiled to [128, NCH, BLOC] (t-major on partitions)
    subt = nc.dram_tensor("subt", [P, NCH, BLOC], F32, kind="ExternalInput").ap()
    rev = nc.dram_tensor("rev", [BLOC, L, T], F32, kind="ExternalInput").ap()
    # wqt: Wq.T, [T, A] row-major
    wqt = nc.dram_tensor("wqt", [T, A], F32, kind="ExternalInput").ap()
    # bqc/bkc: biases chunked [128, NCH] with element (p, c) = bias[128c + p]
    bqc = nc.dram_tensor("bqc", [P, NCH], F32, kind="ExternalInput").ap()
    wk = nc.dram_tensor("wk", [A, T], F32, kind="ExternalInput").ap()
    bkc = nc.dram_tensor("bkc", [P, NCH], F32, kind="ExternalInput").ap()
    ident = nc.dram_tensor("ident", [BLOC, BLOC], F32, kind="ExternalInput").ap()
    out = nc.dram_tensor("out", [BLOC, T], F32, kind="ExternalOutput").ap()

    with tile.TileContext(nc) as tc:
        with (
            tc.tile_pool(name="small", bufs=1) as small,
            tc.tile_pool(name="wqtp", bufs=1) as wqtp,
            tc.tile_pool(name="wkp", bufs=1) as wkp,
            tc.tile_pool(name="qb", bufs=1) as qbp,
            tc.tile_pool(name="rp", bufs=5) as rp,
            tc.tile_pool(name="scr", bufs=3) as scrp,
            tc.tile_pool(name="ep", bufs=4) as ep,
            tc.tile_pool(name="wwp", bufs=2) as wwp,
            tc.tile_pool(name="dram", bufs=1, space="DRAM") as dram,
            tc.tile_pool(name="pstr", bufs=2, space="PSUM") as pstrp,
            tc.tile_pool(name="psq", bufs=1, space="PSUM") as psqp,
            tc.tile_pool(name="psww", bufs=2, space="PSUM") as pswwp,
        ):
            # ---- small loads (already laid out by host) ----
            subt_sb = small.tile([P, NCH, BLOC], F32, name="subt_sb", tag="subt_sb")
            nc.sync.dma_start(out=subt_sb, in_=subt)
            bq_sb = small.tile([P, NCH], F32, name="bq_sb", tag="bq_sb")
            nc.sync.dma_start(out=bq_sb, in_=bqc)
            bk_sb = small.tile([P, NCH], F32, name="bk_sb", tag="bk_sb")
            nc.sync.dma_start(out=bk_sb, in_=bkc)
            id_sb = small.tile([BLOC, BLOC], F32, name="id_sb", tag="id_sb")
            nc.sync.dma_start(out=id_sb, in_=ident)

            # ---- weight loads: wqT[j] = [128 t, 1024 a], wk[i] = [128 a, 1024 t] ----
            wqT = [
                wqtp.tile([P, A], F32, name=f"wqT{j}", tag=f"wqT{j}")
                for j in range(NCH)
            ]
            for j in range(NCH):
                nc.sync.dma_start(out=wqT[j], in_=wqt[j * P : (j + 1) * P, :])
            wk_sb = [
                wkp.tile([P, T], F32, name=f"wk{i}", tag=f"wk{i}") for i in range(NCH)
            ]
            for i in range(NCH):
                nc.sync.dma_start(out=wk_sb[i], in_=wk[i * P : (i + 1) * P, :])

            # ---- Q[b, a] = s @ Wq.T : Wq.T streams as the moving operand,
            #      so these matmuls overlap the weight-DMA wave chunk by chunk.
            psQ = psqp.tile([36, A], F32, name="psQ", tag="psq")[0:BLOC, :]
            for j in range(NCH):
                for h in range(2):
                    nc.tensor.matmul(
                        psQ[:, h * 512 : (h + 1) * 512],
                        subt_sb[:, j, :],
                        wqT[j][:, h * 512 : (h + 1) * 512],
                        start=(j == 0),
                        stop=(j == NCH - 1),
                    )
            Q_sb = small.tile([BLOC, A], F32, name="Q_sb", tag="Q_sb")
            nc.scalar.copy(Q_sb, psQ)

            # ---- QT chunks [128 a, BLOC] via tiny PE transposes; add bq here ----
            QT = small.tile([P, NCH, BLOC], F32, name="QT", tag="QT")
            for i in range(NCH):
                pstr = pstrp.tile([P, BLOC], F32, name="pstr", tag="pstr")
                nc.tensor.transpose(pstr, Q_sb[:, i * P : (i + 1) * P], id_sb)
                nc.scalar.activation(
                    QT[:, i, :], pstr, FT.Identity, bias=bq_sb[:, i : i + 1]
                )

            # ---- qt[b, t] = Q @ Wk on two concurrent PE col-groups;
            #      cb = Q . bk in a separate PSUM bank ----
            psq = psqp.tile([36, T], F32, name="psq", tag="psq")
            cb_ps = pstrp.tile([BLOC, 1], F32, name="cb_ps", tag="pstr")
            for i in range(NCH):
                nc.tensor.matmul(
                    psq[0:BLOC, 0:512],
                    QT[:, i, :],
                    wk_sb[i][:, 0:512],
                    start=(i == 0),
                    stop=(i == NCH - 1),
                    tile_position=(0, 0),
                )
                nc.tensor.matmul(
                    psq[32 : 32 + BLOC, 512:1024],
                    QT[:, i, :],
                    wk_sb[i][:, 512:1024],
                    start=(i == 0),
                    stop=(i == NCH - 1),
                    tile_position=(0, 32),
                )
                nc.tensor.matmul(
                    cb_ps,
                    QT[:, i, :],
                    bk_sb[:, i : i + 1],
                    start=(i == 0),
                    stop=(i == NCH - 1),
                )
            # fold 1/sqrt(A) here; halves sit on partitions 0-3 / 32-35 and
            # get re-joined by the DMA gather into qdram below
            qts0 = small.tile([BLOC, 512], F32, name="qts0", tag="qts0")
            nc.scalar.mul(qts0, psq[0:BLOC, 0:512], SCALE)
            qts1 = small.tile([36, 512], F32, name="qts1", tag="qts1")
            nc.scalar.mul(qts1[32 : 32 + BLOC, :], psq[32 : 32 + BLOC, 512:1024], SCALE)
            qtc = small.tile([BLOC, 1], F32, name="qtc", tag="qtc")
            nc.scalar.mul(qtc, cb_ps, SCALE)

            # ---- broadcast qt rows to 128 partitions via DRAM round-trip ----
            qdram = dram.tile([BLOC, T + 1], F32, name="qdram", tag="qdram")
            nc.gpsimd.dma_start(out=qdram[:, 0:512], in_=qts0)
            nc.gpsimd.dma_start(out=qdram[:, 512:1024], in_=qts1[32 : 32 + BLOC, :])
            nc.gpsimd.dma_start(out=qdram[:, 1024 : T + 1], in_=qtc)
            qb_all = qbp.tile([P, BLOC, T + 1], F32, name="qb_all", tag="qb_all")
            bcast_dma = nc.gpsimd.dma_start(
                out=qb_all.rearrange("p b t -> p (b t)"),
                in_=qdram.rearrange("b t -> (b t)").partition_broadcast(P),
            )
            qb_t = [qb_all[:, b, :] for b in range(BLOC)]

            # ---- main stream: e = (r . qt) + cb ; ww += e.T-weighted rows ----
            rt_dmas = []  # chain reviewer DMAs depth-3 so they complete in
            # order (unchained, the round-robin queues finish the whole
            # first wave together, gating the first compute tile)
            for b in range(BLOC):
                # ww halves accumulate on two concurrent PE col-groups:
                # half 0 -> psum row 0 cols 0:512 (bank 0), half 1 -> psum
                # row 32 cols 512:1024 (bank 1); one start/stop pair each.
                ps_ww = pswwp.tile([33, T], F32, name="ps_ww", tag="ps_ww")
                for ti in range(NTI):
                    rt = rp.tile([P, LTILE, T], F32, name="rt", tag="rt")
                    d = nc.sync.dma_start(
                        out=rt,
                        in_=rev[
                            b, ti * ROWS_PER_TILE : (ti + 1) * ROWS_PER_TILE, :
                        ].rearrange("(p f) t -> p f t", f=LTILE),
                    )
                    if len(rt_dmas) >= 3:
                        add_dep_helper(
                            d.ins, rt_dmas[-3].ins, reason="stagger rt stream"
                        )
                    if 3 <= len(rt_dmas) <= 5:
                        add_dep_helper(
                            d.ins, bcast_dma.ins, reason="let qt broadcast through"
                        )
                    rt_dmas.append(d)
                    e_raw = ep.tile([P, LTILE], F32, name="e_raw", tag="e_raw")
                    e_t = ep.tile([P, LTILE], F32, name="e_t", tag="e_t")
                    for i in range(LTILE):
                        # fused multiply + free-dim reduce on DVE:
                        # scr = r * qt_bcast ; e_raw = sum(scr)
                        scr = scrp.tile([P, T], F32, name="scr", tag="scr")
                        nc.vector.scalar_tensor_tensor(
                            out=scr,
                            in0=rt[:, i, :],
                            scalar=1.0,
                            in1=qb_t[b][:, 0:T],
                            op0=OP.bypass,
                            op1=OP.mult,
                            accum_out=e_raw[:, i : i + 1],
                        )
                        # e = e_raw + cb on ScalarE (cb pre-scaled by 1/sqrt(A))
                        nc.scalar.activation(
                            e_t[:, i : i + 1],
                            e_raw[:, i : i + 1],
                            FT.Identity,
                            bias=qb_t[b][:, T : T + 1],
                        )
                        # ww[0, :] += e_slice.T @ r_slice  (e stationary,
                        # reviewer tile streams as the moving operand)
                        for h in range(2):
                            nc.tensor.matmul(
                                ps_ww[
                                    32 * h : 32 * h + 1, h * 512 : (h + 1) * 512
                                ],
                                e_t[:, i : i + 1],
                                rt[:, i, h * 512 : (h + 1) * 512],
                                start=(ti == 0 and i == 0),
                                stop=(ti == NTI - 1 and i == LTILE - 1),
                                tile_position=(0, 32 * h),
                            )
                ww_sb = wwp.tile([33, T], F32, name="ww_sb", tag="ww_sb")
                nc.scalar.copy(ww_sb[0:1, 0:512], ps_ww[0:1, 0:512])
                nc.scalar.copy(ww_sb[32:33, 512:1024], ps_ww[32:33, 512:1024])
                nc.scalar.dma_start(out=out[b : b + 1, 0:512], in_=ww_sb[0:1, 0:512])
                nc.scalar.dma_start(
                    out=out[b : b + 1, 512:1024], in_=ww_sb[32:33, 512:1024]
                )

    nc.compile()
    return nc


_NC = None


def _get_nc():
    global _NC
    if _NC is None:
        _NC = _build()
    return _NC


def _in_maps(submitter_emb, reviewer_emb, Wq, bq, Wk, bk):
    submitter_emb = np.ascontiguousarray(submitter_emb, dtype=np.float32)
    reviewer_emb = np.ascontiguousarray(reviewer_emb, dtype=np.float32)
    # host-side layout prep (no arithmetic): transposes / chunking for DMA
    wqt_np = np.ascontiguousarray(np.asarray(Wq, dtype=np.float32).T)
    wk_np = np.ascontiguousarray(Wk, dtype=np.float32)
    bqc = np.ascontiguousarray(np.asarray(bq, dtype=np.float32).reshape(NCH, P).T)
    bkc = np.ascontiguousarray(np.asarray(bk, dtype=np.float32).reshape(NCH, P).T)
    ident = np.eye(BLOC, dtype=np.float32)

    in_maps = []
    for core in range(NCORES):
        lo, hi = core * BLOC, (core + 1) * BLOC
        # [BLOC, T] -> [128 p, NCH c, BLOC b] with t = 128*c + p
        subt = np.ascontiguousarray(
            submitter_emb[lo:hi].T.reshape(NCH, P, BLOC).transpose(1, 0, 2)
        )
        in_maps.append(
            {
                "subt": subt,
                "rev": reviewer_emb[lo:hi],
                "wqt": wqt_np,
                "bqc": bqc,
                "wk": wk_np,
                "bkc": bkc,
                "ident": ident,
            }
        )
    return in_maps


def kernel(
    submitter_emb: np.ndarray,
    reviewer_emb: np.ndarray,
    Wq: np.ndarray,
    bq: np.ndarray,
    Wk: np.ndarray,
    bk: np.ndarray,
) -> np.ndarray:
    nc = _get_nc()
    in_maps = _in_maps(submitter_emb, reviewer_emb, Wq, bq, Wk, bk)
    res = run_bass_kernel_spmd(nc, in_maps, core_ids=list(range(NCORES)))
    return np.concatenate([res.results[c]["out"] for c in range(NCORES)], axis=0)
